# revision 14
# baseline (speedup 1.0000x reference)
"""Trainium2 Bass kernel for nn_CalibrationAwareLoss.

Strategy (pure data parallel, 1 image per NeuronCore):
  - ACT (ScalarE, natural_log_exp table): sp(-x) = Ln(1+Exp(-x)),
    p16 = Exp(-sp(-x)) as bf16, and K_ACT threshold Sign passes that
    produce +-1 masks (bf16) with fused count accumulation.
  - DVE: bf16 d16 = p16-y16 with fused sum, remaining threshold masks via
    fused is_ge*ones stt with count accumulation, w = 1/(u+1e-6) via
    reciprocal_approx_fast, seg-loss partial sums via affine_mul_reduce.
  - PE (TensorE): per-threshold masked sums  sum(d * mask)  computed as
    diag(d_blk^T @ mask_blk) accumulated in PSUM over all 128-col blocks;
    diagonals extracted with one fused stt (mult identity + accum) each.
  - GPSIMD: t1 = (y-1)*x and (a+e) elementwise products for the seg /
    evidential terms.
Each core DMAs a [128, 128] f32 tile of raw partial sums; the host
combines them in float64 (cheap O(100) work) into the 4 scalar outputs.
"""
import sys

if "/opt/trn_rl_repo" not in sys.path:
    sys.path.insert(0, "/opt/trn_rl_repo")

import numpy as np

import concourse.bacc as bacc
import concourse.mybir as mybir
import concourse.tile as tile
from concourse.bass_utils import run_bass_kernel_spmd

f32 = mybir.dt.float32
bf16 = mybir.dt.bfloat16
Alu = mybir.AluOpType
Act = mybir.ActivationFunctionType

# ---------------- problem constants (hardcoded per spec) ----------------
B, H, W = 8, 1024, 1024
P = 128
IMG_F = H * W // P          # 8192
MAHAL_N = 4096
MAHAL_F = MAHAL_N // P      # 32
N_CORES = 8
LAMBDA_CAL = 1.0
LAMBDA_UNCERT = 0.1

# f32 bit patterns of jnp.linspace(0,1,11)[1:10]
_BOUND_BITS = [0x3DCCCCCD, 0x3E4CCCCD, 0x3E99999A, 0x3ECCCCCD, 0x3F000000,
               0x3F19999A, 0x3F333333, 0x3F4CCCCD, 0x3F666667]
THRESH = [float(np.uint32(b).view(np.float32)) for b in _BOUND_BITS]
NT = len(THRESH)            # 9
# log-space thresholds: p >= b  <=>  spn = softplus(-x) <= -ln(b)
LTHRESH = [float(np.float32(-np.log(np.float64(t)))) for t in THRESH]

# Thresholds handled by ACT Sign passes compare spn (full f32 precision,
# mask = sign(L_i - spn) in {-1,+1}); the rest compare bf16 p16 on DVE
# (mask = (p16 >= b_i) in {0,1}).  ACT gets the middle (densest) bins.
ACT_SIGN = [2, 3, 4, 5, 6]
DVE_MASK = [i for i in range(NT) if i not in ACT_SIGN]

CHUNK_F = 1024
# accumulator column layout, per chunk: [sum_d, A, B, sum_a, sum_e, t0..t8]
CPC = 5 + NT                # 14 columns per chunk
ACC_COLS = 128


def build_nc(img_f=IMG_F, chunk_f=CHUNK_F, mahal_f=MAHAL_F):
    n_chunk = img_f // chunk_f
    nblk = chunk_f // 128
    assert CPC * n_chunk + NT + 1 <= ACC_COLS

    nc = bacc.Bacc("TRN2", target_bir_lowering=False, debug=False,
                   num_devices=N_CORES)
    pm = nc.dram_tensor("pm", [P, img_f], f32, kind="ExternalInput")
    pl = nc.dram_tensor("pl", [P, img_f], f32, kind="ExternalInput")
    tu = nc.dram_tensor("tu", [P, img_f], f32, kind="ExternalInput")
    au = nc.dram_tensor("au", [P, img_f], f32, kind="ExternalInput")
    eu = nc.dram_tensor("eu", [P, img_f], f32, kind="ExternalInput")
    mh = nc.dram_tensor("mh", [P, mahal_f], f32, kind="ExternalInput")
    ident = nc.dram_tensor("ident", [P, 128], f32, kind="ExternalInput")
    acc_out = nc.dram_tensor("acc", [P, ACC_COLS], f32, kind="ExternalOutput")

    with tile.TileContext(nc) as tc:
        with (
            tc.tile_pool(name="io", bufs=2) as io,
            tc.tile_pool(name="work", bufs=2) as work,
            tc.tile_pool(name="bfp", bufs=2) as bfp,
            tc.tile_pool(name="stat", bufs=1) as stat,
            tc.tile_pool(name="psum", bufs=1, space="PSUM") as psp,
        ):
            # persistent tiles
            acc = stat.tile([P, ACC_COLS], f32, tag="acc")
            nc.vector.memset(acc[:], 0.0)
            id_t = stat.tile([P, 128], f32, tag="ident")
            nc.sync.dma_start(id_t[:], ident[:])
            zeros_m = stat.tile([P, mahal_f], f32, tag="zeros_m")
            nc.gpsimd.memset(zeros_m[:], 0.0)
            bias_t = stat.tile([P, NT], f32, tag="bias")
            for i in range(NT):
                nc.gpsimd.memset(bias_t[:, i : i + 1], LTHRESH[i])

            # PSUM tiles are padded to a full bank (2KB/partition); pack 4
            # threshold accumulators per bank as 128-col slices.
            n_banks = (NT + 3) // 4
            ps_banks = [psp.tile([128, min(512, (NT - 4 * k) * 128)], f32,
                                 tag=f"psb{k}", name=f"psb{k}")
                        for k in range(n_banks)]
            ps = [ps_banks[i // 4][:, (i % 4) * 128:(i % 4) * 128 + 128]
                  for i in range(NT)]
            for k in range(n_banks):
                nc.vector.memset(ps_banks[k][:], 0.0)

            # mahal: ood partial = sum(relu(mh - 2))
            mh_t = stat.tile([P, mahal_f], f32, tag="mh")
            nc.sync.dma_start(mh_t[:], mh[:])
            scr_m = stat.tile([P, mahal_f], f32, tag="scr_m")
            nc.vector.scalar_tensor_tensor(
                out=scr_m[:], in0=mh_t[:], scalar=2.0, in1=zeros_m[:],
                op0=Alu.subtract, op1=Alu.max,
                accum_out=acc[:, CPC * n_chunk + NT : CPC * n_chunk + NT + 1])

            for c in range(n_chunk):
                sl = slice(c * chunk_f, (c + 1) * chunk_f)
                col = c * CPC

                x = io.tile([P, chunk_f], f32, tag="x")
                y = io.tile([P, chunk_f], f32, tag="y")
                u = io.tile([P, chunk_f], f32, tag="u")
                a = io.tile([P, chunk_f], f32, tag="a")
                e = io.tile([P, chunk_f], f32, tag="e")
                nc.sync.dma_start(x[:], pm[:, sl])
                nc.sync.dma_start(y[:], pl[:, sl])
                nc.sync.dma_start(u[:], tu[:, sl])
                nc.sync.dma_start(a[:], au[:, sl])
                nc.sync.dma_start(e[:], eu[:, sl])

                # ---- ACT chain: e1=Exp(-x); spn=Ln(e1+1); p16=Exp(-spn)
                spn = work.tile([P, chunk_f], f32, tag="spn")
                nc.scalar.activation(spn[:], x[:], Act.Exp, scale=-1.0)
                nc.scalar.activation(spn[:], spn[:], Act.Ln, bias=1.0)
                p16 = bfp.tile([P, chunk_f], bf16, tag="p16")
                nc.scalar.activation(p16[:], spn[:], Act.Exp, scale=-1.0)

                # ---- d16 = p16 - y16 (bf16) with fused sum(d)
                # (tensor_tensor_reduce is broken on HW; stt works)
                y16 = bfp.tile([P, chunk_f], bf16, tag="y16")
                nc.vector.tensor_copy(y16[:], y[:])
                d16 = bfp.tile([P, chunk_f], bf16, tag="d16")
                nc.vector.scalar_tensor_tensor(
                    out=d16[:], in0=p16[:], scalar=0.0, in1=y16[:],
                    op0=Alu.add, op1=Alu.subtract,
                    accum_out=acc[:, col : col + 1])

                # ---- threshold masks (+ fused count/sign sums)
                masks = [None] * NT
                for i in ACT_SIGN:
                    s = bfp.tile([P, chunk_f], bf16, tag=f"s{i}")
                    nc.scalar.activation(
                        s[:], spn[:], Act.Sign, scale=-1.0,
                        bias=bias_t[:, i : i + 1],
                        accum_out=acc[:, col + 5 + i : col + 6 + i])
                    masks[i] = s
                for i in DVE_MASK:
                    # exact mask: p >= b_i  <=>  spn < L_i ; fused count,
                    # bf16 {0,1} mask out, fast single-src 2x mode
                    s = bfp.tile([P, chunk_f], bf16, tag=f"s{i}")
                    nc.vector.tensor_scalar(
                        out=s[:], in0=spn[:], scalar1=LTHRESH[i], scalar2=None,
                        op0=Alu.is_lt, op1=Alu.add,
                        accum_out=acc[:, col + 5 + i : col + 6 + i])
                    masks[i] = s

                # ---- PE: psum_i += d_blk^T @ mask_blk  (diag = masked sums)
                for b in range(nblk):
                    bs = slice(b * 128, (b + 1) * 128)
                    last = c == n_chunk - 1 and b == nblk - 1
                    for i in range(NT):
                        nc.tensor.matmul(ps[i], d16[:, bs], masks[i][:, bs],
                                         start=False, stop=last,
                                         skip_group_check=True)

                # ---- seg loss partials
                nc.vector.tensor_scalar(out=u[:], in0=u[:], scalar1=1e-6,
                                        scalar2=None, op0=Alu.add)
                w = work.tile([P, chunk_f], f32, tag="w")
                nc.vector.reciprocal_approx_fast(out=w[:], in_=u[:])
                scrA = work.tile([P, chunk_f], f32, tag="scrA")
                nc.vector.affine_mul_reduce(
                    out=scrA[:], accum_out=acc[:, col + 1 : col + 2],
                    in0=spn[:], in1=w[:], scale=1.0, bias=0.0)
                # t1 = (y-1)*x on GPSIMD
                y1 = work.tile([P, chunk_f], f32, tag="y1")
                nc.gpsimd.tensor_scalar(out=y1[:], in0=y[:], scalar1=1.0,
                                        scalar2=None, op0=Alu.subtract)
                nc.gpsimd.tensor_tensor(out=y1[:], in0=y1[:], in1=x[:],
                                        op=Alu.mult)
                scrB = work.tile([P, chunk_f], f32, tag="scrB")
                nc.vector.affine_mul_reduce(
                    out=scrB[:], accum_out=acc[:, col + 2 : col + 3],
                    in0=y1[:], in1=w[:], scale=1.0, bias=0.0)

                # ---- evidential: sum(a) and sum(e)
                ae = work.tile([P, chunk_f], f32, tag="ae")
                nc.gpsimd.tensor_tensor(out=ae[:], in0=a[:], in1=e[:],
                                        op=Alu.add)
                nc.vector.tensor_scalar(out=ae[:], in0=ae[:], scalar1=0.0,
                                        scalar2=None, op0=Alu.add, op1=Alu.add,
                                        accum_out=acc[:, col + 3 : col + 4])

            # ---- diag extraction: acc <- sum_n ps[i][m,n]*I[m,n]
            for i in range(NT):
                scr_d = stat.tile([P, 128], f32, tag=f"scrd{i}")
                nc.vector.scalar_tensor_tensor(
                    out=scr_d[:], in0=ps[i], scalar=1.0, in1=id_t[:],
                    op0=Alu.mult, op1=Alu.mult,
                    accum_out=acc[:, CPC * n_chunk + i : CPC * n_chunk + i + 1])

            nc.sync.dma_start(acc_out[:], acc[:])

    nc.compile()
    return nc


# ---------------- host-side combine ----------------

def combine_host(acc_list, img_f=IMG_F, chunk_f=CHUNK_F, mahal_n=MAHAL_N):
    """acc_list: per-core [128, ACC_COLS] f32 arrays -> 4 f32 scalars."""
    n_chunk = img_f // chunk_f
    n_img = float(P * img_f)
    seg_num = 0.0
    evid_num = 0.0
    cal_imgs = []
    ood_imgs = []
    for arr in acc_list:
        cs = arr.astype(np.float64).sum(axis=0)   # [ACC_COLS]
        sum_d = 0.0
        A = 0.0
        Bv = 0.0
        sum_ae = 0.0
        tsum = np.zeros(NT)
        for c in range(n_chunk):
            col = c * CPC
            sum_d += cs[col]
            A += cs[col + 1]
            Bv += cs[col + 2]
            sum_ae += cs[col + 3]
            tsum += cs[col + 5 : col + 5 + NT]
        diag = cs[CPC * n_chunk : CPC * n_chunk + NT]
        ood = cs[CPC * n_chunk + NT]

        # cumulative counts / d-sums per threshold
        ccount = np.empty(NT + 2)
        dcum = np.empty(NT + 2)
        ccount[0] = n_img
        dcum[0] = sum_d
        for i in range(NT):
            if i in ACT_SIGN:
                ccount[i + 1] = (tsum[i] + n_img) / 2.0
                dcum[i + 1] = (diag[i] + sum_d) / 2.0
            else:
                ccount[i + 1] = tsum[i]
                dcum[i + 1] = diag[i]
        ccount[NT + 1] = 0.0
        dcum[NT + 1] = 0.0

        n_bin = ccount[:-1] - ccount[1:]          # [10]
        d_bin = dcum[:-1] - dcum[1:]
        valid = n_bin > 0
        safe = np.where(valid, n_bin, 1.0)
        err = np.abs(d_bin / safe)
        n_valid = valid.sum()
        cal_imgs.append((err * valid).sum() / max(n_valid, 1.0)
                        if n_valid > 0 else 0.0)

        seg_num += A - Bv
        evid_num += sum_ae
        ood_imgs.append(ood / float(mahal_n))

    n_cores = len(acc_list)
    seg = seg_num / (n_cores * n_img)
    cal = float(np.mean(cal_imgs))
    uncert = float(np.mean(ood_imgs)) + evid_num / (n_cores * n_img)
    total = seg + LAMBDA_CAL * cal + LAMBDA_UNCERT * uncert
    return np.array([total, seg, cal, uncert], dtype=np.float32)


# ---------------- public entry point ----------------

_NC_CACHE = {}


def _get_nc():
    if "nc" not in _NC_CACHE:
        _NC_CACHE["nc"] = build_nc()
    return _NC_CACHE["nc"]


def kernel(pred_masks, pseudo_labels, total_uncertainty,
           aleatoric_uncertainty, epistemic_uncertainty, mahal_distances):
    pm = np.ascontiguousarray(np.asarray(pred_masks, dtype=np.float32))
    pl = np.ascontiguousarray(np.asarray(pseudo_labels, dtype=np.float32))
    tu = np.ascontiguousarray(np.asarray(total_uncertainty, dtype=np.float32))
    au = np.ascontiguousarray(np.asarray(aleatoric_uncertainty, dtype=np.float32))
    eu = np.ascontiguousarray(np.asarray(epistemic_uncertainty, dtype=np.float32))
    mh = np.ascontiguousarray(np.asarray(mahal_distances, dtype=np.float32))

    nc = _get_nc()
    eye = np.eye(P, dtype=np.float32)
    in_maps = []
    for b in range(N_CORES):
        in_maps.append({
            "pm": pm[b].reshape(P, IMG_F),
            "pl": pl[b].reshape(P, IMG_F),
            "tu": tu[b].reshape(P, IMG_F),
            "au": au[b].reshape(P, IMG_F),
            "eu": eu[b].reshape(P, IMG_F),
            "mh": mh[b].reshape(P, MAHAL_F),
            "ident": eye,
        })
    res = run_bass_kernel_spmd(nc, in_maps, core_ids=list(range(N_CORES)))
    return combine_host([res.results[b]["acc"] for b in range(N_CORES)])


# revision 19
# speedup vs baseline: 1.7573x; 1.7573x over previous
"""Trainium2 Bass kernel for nn_CalibrationAwareLoss.

Strategy (pure data parallel, 1 image per NeuronCore):
  - ACT (ScalarE, natural_log_exp table): sp(-x) = Ln(1+Exp(-x)),
    p16 = Exp(-sp(-x)) as bf16, and K_ACT threshold Sign passes that
    produce +-1 masks (bf16) with fused count accumulation.
  - DVE: bf16 d16 = p16-y16 with fused sum, remaining threshold masks via
    fused is_ge*ones stt with count accumulation, w = 1/(u+1e-6) via
    reciprocal_approx_fast, seg-loss partial sums via affine_mul_reduce.
  - PE (TensorE): per-threshold masked sums  sum(d * mask)  computed as
    diag(d_blk^T @ mask_blk) accumulated in PSUM over all 128-col blocks;
    diagonals extracted with one fused stt (mult identity + accum) each.
  - GPSIMD: t1 = (y-1)*x and (a+e) elementwise products for the seg /
    evidential terms.
Each core DMAs a [128, 128] f32 tile of raw partial sums; the host
combines them in float64 (cheap O(100) work) into the 4 scalar outputs.
"""
import sys

if "/opt/trn_rl_repo" not in sys.path:
    sys.path.insert(0, "/opt/trn_rl_repo")

import numpy as np

import concourse.bacc as bacc
import concourse.mybir as mybir
import concourse.tile as tile
from concourse.bass_utils import run_bass_kernel_spmd

# Pin every activation to the natural_log_exp_and_others table (has Exp, Ln,
# Sign, Copy, Identity, MemsetZero) so the table-load pass emits one load
# instead of thrashing between tables (~1.3us per reload on HW).
_orig_get_tables = bacc.get_activation_tables


def _pinned_tables(arch):
    t = _orig_get_tables(arch)
    return {name: (funcs if name == "natural_log_exp_and_others" else set())
            for name, funcs in t.items()}


bacc.get_activation_tables = _pinned_tables

f32 = mybir.dt.float32
bf16 = mybir.dt.bfloat16
Alu = mybir.AluOpType
Act = mybir.ActivationFunctionType

# ---------------- problem constants (hardcoded per spec) ----------------
B, H, W = 8, 1024, 1024
P = 128
IMG_F = H * W // P          # 8192
MAHAL_N = 4096
MAHAL_F = MAHAL_N // P      # 32
N_CORES = 8
LAMBDA_CAL = 1.0
LAMBDA_UNCERT = 0.1

# f32 bit patterns of jnp.linspace(0,1,11)[1:10]
_BOUND_BITS = [0x3DCCCCCD, 0x3E4CCCCD, 0x3E99999A, 0x3ECCCCCD, 0x3F000000,
               0x3F19999A, 0x3F333333, 0x3F4CCCCD, 0x3F666667]
THRESH = [float(np.uint32(b).view(np.float32)) for b in _BOUND_BITS]
NT = len(THRESH)            # 9
# log-space thresholds: p >= b  <=>  spn = softplus(-x) <= -ln(b)
LTHRESH = [float(np.float32(-np.log(np.float64(t)))) for t in THRESH]

# Thresholds handled by ACT Sign passes compare spn (full f32 precision,
# mask = sign(L_i - spn) in {-1,+1}); the rest compare bf16 p16 on DVE
# (mask = (p16 >= b_i) in {0,1}).  ACT gets the middle (densest) bins.
ACT_SIGN = [2, 3, 4, 5, 6]
DVE_MASK = [i for i in range(NT) if i not in ACT_SIGN]

CHUNK_F = 1024
# accumulator column layout, per chunk: [sum_d, A, B, sum_a, sum_e, t0..t8]
CPC = 5 + NT                # 14 columns per chunk
ACC_COLS = 128


def build_nc(img_f=IMG_F, chunk_f=CHUNK_F, mahal_f=MAHAL_F):
    n_chunk = img_f // chunk_f
    nblk = chunk_f // 128
    assert CPC * n_chunk + NT + 1 <= ACC_COLS

    nc = bacc.Bacc("TRN2", target_bir_lowering=False, debug=False,
                   num_devices=N_CORES)
    pm = nc.dram_tensor("pm", [P, img_f], f32, kind="ExternalInput")
    pl = nc.dram_tensor("pl", [P, img_f], f32, kind="ExternalInput")
    tu = nc.dram_tensor("tu", [P, img_f], f32, kind="ExternalInput")
    au = nc.dram_tensor("au", [P, img_f], f32, kind="ExternalInput")
    eu = nc.dram_tensor("eu", [P, img_f], f32, kind="ExternalInput")
    mh = nc.dram_tensor("mh", [P, mahal_f], f32, kind="ExternalInput")
    ident = nc.dram_tensor("ident", [P, 128], f32, kind="ExternalInput")
    acc_out = nc.dram_tensor("acc", [P, ACC_COLS], f32, kind="ExternalOutput")

    with tile.TileContext(nc) as tc:
        with (
            tc.tile_pool(name="io", bufs=2) as io,
            tc.tile_pool(name="work", bufs=2) as work,
            tc.tile_pool(name="bfp", bufs=2) as bfp,
            tc.tile_pool(name="stat", bufs=1) as stat,
            tc.tile_pool(name="psum", bufs=1, space="PSUM") as psp,
        ):
            # persistent tiles
            acc = stat.tile([P, ACC_COLS], f32, tag="acc")
            nc.vector.memset(acc[:], 0.0)
            id_t = stat.tile([P, 128], f32, tag="ident")
            nc.sync.dma_start(id_t[:], ident[:])
            zeros_m = stat.tile([P, mahal_f], f32, tag="zeros_m")
            nc.gpsimd.memset(zeros_m[:], 0.0)
            bias_t = stat.tile([P, NT], f32, tag="bias")
            for i in range(NT):
                nc.gpsimd.memset(bias_t[:, i : i + 1], LTHRESH[i])

            # PSUM tiles are padded to a full bank (2KB/partition); pack 4
            # threshold accumulators per bank as 128-col slices.
            n_banks = (NT + 3) // 4
            ps_banks = [psp.tile([128, min(512, (NT - 4 * k) * 128)], f32,
                                 tag=f"psb{k}", name=f"psb{k}")
                        for k in range(n_banks)]
            for k in range(n_banks):
                nc.vector.memset(ps_banks[k][:], 0.0)

            # mahal: ood partial = sum(relu(mh - 2))
            mh_t = stat.tile([P, mahal_f], f32, tag="mh")
            nc.sync.dma_start(mh_t[:], mh[:])
            scr_m = stat.tile([P, mahal_f], f32, tag="scr_m")
            nc.vector.scalar_tensor_tensor(
                out=scr_m[:], in0=mh_t[:], scalar=2.0, in1=zeros_m[:],
                op0=Alu.subtract, op1=Alu.max,
                accum_out=acc[:, CPC * n_chunk + NT : CPC * n_chunk + NT + 1])

            for c in range(n_chunk):
                sl = slice(c * chunk_f, (c + 1) * chunk_f)
                col = c * CPC

                x = io.tile([P, chunk_f], f32, tag="x")
                y = io.tile([P, chunk_f], f32, tag="y")
                u = io.tile([P, chunk_f], f32, tag="u")
                a = io.tile([P, chunk_f], f32, tag="a")
                e = io.tile([P, chunk_f], f32, tag="e")
                nc.sync.dma_start(x[:], pm[:, sl])
                nc.sync.dma_start(y[:], pl[:, sl])
                nc.sync.dma_start(u[:], tu[:, sl])
                nc.sync.dma_start(a[:], au[:, sl])
                nc.sync.dma_start(e[:], eu[:, sl])

                # ---- ACT chain: e1=Exp(-x); spn=Ln(e1+1); p16=Exp(-spn)
                spn = work.tile([P, chunk_f], f32, tag="spn")
                nc.scalar.activation(spn[:], x[:], Act.Exp, scale=-1.0)
                nc.scalar.activation(spn[:], spn[:], Act.Ln, bias=1.0)
                p16 = bfp.tile([P, chunk_f], bf16, tag="p16")
                nc.scalar.activation(p16[:], spn[:], Act.Exp, scale=-1.0)

                # ---- d16 = p16 - y16 (bf16) with fused sum(d)
                # (tensor_tensor_reduce is broken on HW; stt works)
                y16 = bfp.tile([P, chunk_f], bf16, tag="y16")
                nc.vector.tensor_scalar(out=y16[:], in0=y[:], scalar1=0.0,
                                        scalar2=None, op0=Alu.add)
                d16 = bfp.tile([P, chunk_f], bf16, tag="d16")
                nc.vector.scalar_tensor_tensor(
                    out=d16[:], in0=p16[:], scalar=0.0, in1=y16[:],
                    op0=Alu.add, op1=Alu.subtract,
                    accum_out=acc[:, col : col + 1])

                # ---- threshold masks (+ fused count/sign sums), written
                # block-interleaved per PSUM bank group so one matmul can
                # consume 4 thresholds: mg[k][p, blk, slot, col]
                mg = [bfp.tile([P, nblk, min(4, NT - 4 * k), 128], bf16,
                               tag=f"mg{k}", name=f"mg{k}_{c}")
                      for k in range(n_banks)]
                for i in range(NT):
                    out_ap = mg[i // 4][:, :, i % 4, :]
                    if i in ACT_SIGN:
                        nc.scalar.activation(
                            out_ap, spn[:], Act.Sign, scale=-1.0,
                            bias=bias_t[:, i : i + 1],
                            accum_out=acc[:, col + 5 + i : col + 6 + i])
                    else:
                        # exact mask: p >= b_i  <=>  spn < L_i ; fused count
                        nc.vector.tensor_scalar(
                            out=out_ap, in0=spn[:], scalar1=LTHRESH[i],
                            scalar2=None, op0=Alu.is_lt, op1=Alu.add,
                            accum_out=acc[:, col + 5 + i : col + 6 + i])

                # ---- PE: ps_bank[k] += d_blk^T @ [4 masks]  (diag = sums)
                for b in range(nblk):
                    bs = slice(b * 128, (b + 1) * 128)
                    last = c == n_chunk - 1 and b == nblk - 1
                    for k in range(n_banks):
                        nc.tensor.matmul(ps_banks[k][:],
                                         d16[:, bs],
                                         mg[k][:, b, :, :],
                                         start=False, stop=last,
                                         skip_group_check=True)

                # ---- seg loss partials
                ueps = work.tile([P, chunk_f], f32, tag="ueps")
                nc.vector.tensor_scalar(out=ueps[:], in0=u[:], scalar1=1e-6,
                                        scalar2=None, op0=Alu.add)
                w = work.tile([P, chunk_f], f32, tag="w")
                nc.vector.reciprocal_approx_fast(out=w[:], in_=ueps[:])
                scrA = work.tile([P, chunk_f], f32, tag="scrA")
                nc.vector.affine_mul_reduce(
                    out=scrA[:], accum_out=acc[:, col + 1 : col + 2],
                    in0=spn[:], in1=w[:], scale=1.0, bias=0.0)
                # t1 = (y-1)*x in one fused DVE op (gpsimd ts is ~12x slower)
                y1 = work.tile([P, chunk_f], f32, tag="y1")
                nc.vector.scalar_tensor_tensor(
                    out=y1[:], in0=y[:], scalar=1.0, in1=x[:],
                    op0=Alu.subtract, op1=Alu.mult)
                scrB = work.tile([P, chunk_f], f32, tag="scrB")
                nc.vector.affine_mul_reduce(
                    out=scrB[:], accum_out=acc[:, col + 2 : col + 3],
                    in0=y1[:], in1=w[:], scale=1.0, bias=0.0)

                # ---- evidential: sum(a) and sum(e)
                ae = work.tile([P, chunk_f], f32, tag="ae")
                nc.gpsimd.tensor_tensor(out=ae[:], in0=a[:], in1=e[:],
                                        op=Alu.add)
                nc.vector.tensor_scalar(out=ae[:], in0=ae[:], scalar1=0.0,
                                        scalar2=None, op0=Alu.add, op1=Alu.add,
                                        accum_out=acc[:, col + 3 : col + 4])

            # ---- diag extraction: PSUM -> SBUF via ACT copy (DVE PSUM reads
            # are slow), then acc <- sum_n sb[m,n]*I[m,n]
            for k in range(n_banks):
                w_bank = min(512, (NT - 4 * k) * 128)
                sb = stat.tile([P, w_bank], f32, tag=f"sbps{k}",
                               name=f"sbps{k}")
                nc.scalar.copy(sb[:], ps_banks[k][:])
                for j in range(w_bank // 128):
                    i = 4 * k + j
                    scr_d = stat.tile([P, 128], f32, tag=f"scrd{i}",
                                      name=f"scrd{i}")
                    nc.vector.scalar_tensor_tensor(
                        out=scr_d[:], in0=sb[:, j * 128 : (j + 1) * 128],
                        scalar=1.0, in1=id_t[:], op0=Alu.mult, op1=Alu.mult,
                        accum_out=acc[:, CPC * n_chunk + i :
                                      CPC * n_chunk + i + 1])

            nc.sync.dma_start(acc_out[:], acc[:])

    nc.compile()
    return nc


# ---------------- host-side combine ----------------

def combine_host(acc_list, img_f=IMG_F, chunk_f=CHUNK_F, mahal_n=MAHAL_N):
    """acc_list: per-core [128, ACC_COLS] f32 arrays -> 4 f32 scalars."""
    n_chunk = img_f // chunk_f
    n_img = float(P * img_f)
    seg_num = 0.0
    evid_num = 0.0
    cal_imgs = []
    ood_imgs = []
    for arr in acc_list:
        cs = arr.astype(np.float64).sum(axis=0)   # [ACC_COLS]
        sum_d = 0.0
        A = 0.0
        Bv = 0.0
        sum_ae = 0.0
        tsum = np.zeros(NT)
        for c in range(n_chunk):
            col = c * CPC
            sum_d += cs[col]
            A += cs[col + 1]
            Bv += cs[col + 2]
            sum_ae += cs[col + 3]
            tsum += cs[col + 5 : col + 5 + NT]
        diag = cs[CPC * n_chunk : CPC * n_chunk + NT]
        ood = cs[CPC * n_chunk + NT]

        # cumulative counts / d-sums per threshold
        ccount = np.empty(NT + 2)
        dcum = np.empty(NT + 2)
        ccount[0] = n_img
        dcum[0] = sum_d
        for i in range(NT):
            if i in ACT_SIGN:
                ccount[i + 1] = (tsum[i] + n_img) / 2.0
                dcum[i + 1] = (diag[i] + sum_d) / 2.0
            else:
                ccount[i + 1] = tsum[i]
                dcum[i + 1] = diag[i]
        ccount[NT + 1] = 0.0
        dcum[NT + 1] = 0.0

        n_bin = ccount[:-1] - ccount[1:]          # [10]
        d_bin = dcum[:-1] - dcum[1:]
        valid = n_bin > 0
        safe = np.where(valid, n_bin, 1.0)
        err = np.abs(d_bin / safe)
        n_valid = valid.sum()
        cal_imgs.append((err * valid).sum() / max(n_valid, 1.0)
                        if n_valid > 0 else 0.0)

        seg_num += A - Bv
        evid_num += sum_ae
        ood_imgs.append(ood / float(mahal_n))

    n_cores = len(acc_list)
    seg = seg_num / (n_cores * n_img)
    cal = float(np.mean(cal_imgs))
    uncert = float(np.mean(ood_imgs)) + evid_num / (n_cores * n_img)
    total = seg + LAMBDA_CAL * cal + LAMBDA_UNCERT * uncert
    return np.array([total, seg, cal, uncert], dtype=np.float32)


# ---------------- public entry point ----------------

_NC_CACHE = {}


def _get_nc():
    if "nc" not in _NC_CACHE:
        _NC_CACHE["nc"] = build_nc()
    return _NC_CACHE["nc"]


def kernel(pred_masks, pseudo_labels, total_uncertainty,
           aleatoric_uncertainty, epistemic_uncertainty, mahal_distances):
    pm = np.ascontiguousarray(np.asarray(pred_masks, dtype=np.float32))
    pl = np.ascontiguousarray(np.asarray(pseudo_labels, dtype=np.float32))
    tu = np.ascontiguousarray(np.asarray(total_uncertainty, dtype=np.float32))
    au = np.ascontiguousarray(np.asarray(aleatoric_uncertainty, dtype=np.float32))
    eu = np.ascontiguousarray(np.asarray(epistemic_uncertainty, dtype=np.float32))
    mh = np.ascontiguousarray(np.asarray(mahal_distances, dtype=np.float32))

    nc = _get_nc()
    eye = np.eye(P, dtype=np.float32)
    in_maps = []
    for b in range(N_CORES):
        in_maps.append({
            "pm": pm[b].reshape(P, IMG_F),
            "pl": pl[b].reshape(P, IMG_F),
            "tu": tu[b].reshape(P, IMG_F),
            "au": au[b].reshape(P, IMG_F),
            "eu": eu[b].reshape(P, IMG_F),
            "mh": mh[b].reshape(P, MAHAL_F),
            "ident": eye,
        })
    res = run_bass_kernel_spmd(nc, in_maps, core_ids=list(range(N_CORES)))
    return combine_host([res.results[b]["acc"] for b in range(N_CORES)])


# revision 27
# speedup vs baseline: 1.8570x; 1.0567x over previous
"""Trainium2 Bass kernel for nn_CalibrationAwareLoss.

Strategy (pure data parallel, 1 image per NeuronCore):
  - ACT (ScalarE, natural_log_exp table): sp(-x) = Ln(1+Exp(-x)),
    p16 = Exp(-sp(-x)) as bf16, and K_ACT threshold Sign passes that
    produce +-1 masks (bf16) with fused count accumulation.
  - DVE: bf16 d16 = p16-y16 with fused sum, remaining threshold masks via
    fused is_ge*ones stt with count accumulation, w = 1/(u+1e-6) via
    reciprocal_approx_fast, seg-loss partial sums via affine_mul_reduce.
  - PE (TensorE): per-threshold masked sums  sum(d * mask)  computed as
    diag(d_blk^T @ mask_blk) accumulated in PSUM over all 128-col blocks;
    diagonals extracted with one fused stt (mult identity + accum) each.
  - GPSIMD: t1 = (y-1)*x and (a+e) elementwise products for the seg /
    evidential terms.
Each core DMAs a [128, 128] f32 tile of raw partial sums; the host
combines them in float64 (cheap O(100) work) into the 4 scalar outputs.
"""
import sys

if "/opt/trn_rl_repo" not in sys.path:
    sys.path.insert(0, "/opt/trn_rl_repo")

import numpy as np

import concourse.bacc as bacc
import concourse.mybir as mybir
import concourse.tile as tile
from concourse.bass_utils import run_bass_kernel_spmd

# Pin every activation to the natural_log_exp_and_others table (has Exp, Ln,
# Sign, Copy, Identity, MemsetZero) so the table-load pass emits one load
# instead of thrashing between tables (~1.3us per reload on HW).
_orig_get_tables = bacc.get_activation_tables


def _pinned_tables(arch):
    t = _orig_get_tables(arch)
    return {name: (funcs if name == "natural_log_exp_and_others" else set())
            for name, funcs in t.items()}


bacc.get_activation_tables = _pinned_tables

f32 = mybir.dt.float32
bf16 = mybir.dt.bfloat16
Alu = mybir.AluOpType
Act = mybir.ActivationFunctionType

# ---------------- problem constants (hardcoded per spec) ----------------
B, H, W = 8, 1024, 1024
P = 128
IMG_F = H * W // P          # 8192
MAHAL_N = 4096
MAHAL_F = MAHAL_N // P      # 32
N_CORES = 8
LAMBDA_CAL = 1.0
LAMBDA_UNCERT = 0.1

# f32 bit patterns of jnp.linspace(0,1,11)[1:10]
_BOUND_BITS = [0x3DCCCCCD, 0x3E4CCCCD, 0x3E99999A, 0x3ECCCCCD, 0x3F000000,
               0x3F19999A, 0x3F333333, 0x3F4CCCCD, 0x3F666667]
THRESH = [float(np.uint32(b).view(np.float32)) for b in _BOUND_BITS]
NT = len(THRESH)            # 9
# log-space thresholds: p >= b  <=>  spn = softplus(-x) <= -ln(b)
LTHRESH = [float(np.float32(-np.log(np.float64(t)))) for t in THRESH]

# Thresholds handled by ACT Sign passes compare spn (full f32 precision,
# mask = sign(L_i - spn) in {-1,+1}); the rest compare spn on DVE
# (mask = (spn < L_i) in {0,1}).  ACT gets the middle (densest) bins.
ACT_SIGN = [2, 3, 4, 5, 6]
DVE_MASK = [i for i in range(NT) if i not in ACT_SIGN]
# PSUM bank grouping: DVE-mask thresholds share bank 0 so ONE extra
# ones-matmul stream per block yields all their counts (the DVE mask ops
# then skip the slow fused-reduce path); ACT thresholds' counts ride free
# on their activation accum_out.
GROUP_ORDER = DVE_MASK + ACT_SIGN           # threshold index by (bank,slot)
G_OF = {i: g for g, i in enumerate(GROUP_ORDER)}

CHUNK_F = 1024
# accumulator column layout, per chunk: [sum_d, A, B, sum_a, sum_e, t0..t8];
# image-level tail: diag[NT], ood, dve-group counts[len(DVE_MASK)]
CPC = 5 + NT                # 14 columns per chunk
ACC_COLS = 128


def build_nc(img_f=IMG_F, chunk_f=CHUNK_F, mahal_f=MAHAL_F):
    n_chunk = img_f // chunk_f
    nblk = chunk_f // 128
    CNT_COL = CPC * n_chunk + NT + 1
    assert CNT_COL + len(DVE_MASK) <= ACC_COLS

    nc = bacc.Bacc("TRN2", target_bir_lowering=False, debug=False,
                   num_devices=N_CORES)
    pm = nc.dram_tensor("pm", [P, img_f], f32, kind="ExternalInput")
    pl = nc.dram_tensor("pl", [P, img_f], f32, kind="ExternalInput")
    tu = nc.dram_tensor("tu", [P, img_f], f32, kind="ExternalInput")
    au = nc.dram_tensor("au", [P, img_f], f32, kind="ExternalInput")
    eu = nc.dram_tensor("eu", [P, img_f], f32, kind="ExternalInput")
    mh = nc.dram_tensor("mh", [P, mahal_f], f32, kind="ExternalInput")
    ident = nc.dram_tensor("ident", [P, 128], f32, kind="ExternalInput")
    acc_out = nc.dram_tensor("acc", [P, ACC_COLS], f32, kind="ExternalOutput")

    with tile.TileContext(nc) as tc:
        with (
            tc.tile_pool(name="io", bufs=2) as io,
            tc.tile_pool(name="work", bufs=2) as work,
            tc.tile_pool(name="bfp", bufs=2) as bfp,
            tc.tile_pool(name="stat", bufs=1) as stat,
            tc.tile_pool(name="psum", bufs=1, space="PSUM") as psp,
        ):
            # persistent tiles
            acc = stat.tile([P, ACC_COLS], f32, tag="acc")
            nc.vector.memset(acc[:], 0.0)
            id_t = stat.tile([P, 128], f32, tag="ident")
            nc.sync.dma_start(id_t[:], ident[:])
            zeros_m = stat.tile([P, mahal_f], f32, tag="zeros_m")
            nc.gpsimd.memset(zeros_m[:], 0.0)
            bias_t = stat.tile([P, NT], f32, tag="bias")
            for i in range(NT):
                nc.gpsimd.memset(bias_t[:, i : i + 1], LTHRESH[i])

            # PSUM tiles are padded to a full bank (2KB/partition); pack 4
            # threshold accumulators per bank as 128-col slices.
            n_banks = (NT + 3) // 4
            ps_banks = [psp.tile([128, min(512, (NT - 4 * k) * 128)], f32,
                                 tag=f"psb{k}", name=f"psb{k}")
                        for k in range(n_banks)]
            for k in range(n_banks):
                nc.vector.memset(ps_banks[k][:], 0.0)
            # count accumulator bank for the DVE-mask group (bank 0)
            n_cnt = len(DVE_MASK)
            ps_cnt = psp.tile([128, n_cnt * 128], f32, tag="pscnt",
                              name="pscnt")
            nc.vector.memset(ps_cnt[:], 0.0)
            ones16 = stat.tile([P, 128], bf16, tag="ones16")
            nc.gpsimd.memset(ones16[:], 1.0)

            # mahal: ood partial = sum(relu(mh - 2))
            mh_t = stat.tile([P, mahal_f], f32, tag="mh")
            nc.sync.dma_start(mh_t[:], mh[:])
            scr_m = stat.tile([P, mahal_f], f32, tag="scr_m")
            nc.vector.scalar_tensor_tensor(
                out=scr_m[:], in0=mh_t[:], scalar=2.0, in1=zeros_m[:],
                op0=Alu.subtract, op1=Alu.max,
                accum_out=acc[:, CPC * n_chunk + NT : CPC * n_chunk + NT + 1])

            for c in range(n_chunk):
                sl = slice(c * chunk_f, (c + 1) * chunk_f)
                col = c * CPC

                x = io.tile([P, chunk_f], f32, tag="x")
                y = io.tile([P, chunk_f], f32, tag="y")
                u = io.tile([P, chunk_f], f32, tag="u")
                a = io.tile([P, chunk_f], f32, tag="a")
                e = io.tile([P, chunk_f], f32, tag="e")
                nc.sync.dma_start(x[:], pm[:, sl])
                nc.sync.dma_start(y[:], pl[:, sl])
                nc.sync.dma_start(u[:], tu[:, sl])
                nc.sync.dma_start(a[:], au[:, sl])
                nc.sync.dma_start(e[:], eu[:, sl])

                # ---- ACT chain: e1=Exp(-x); spn=Ln(e1+1); p16=Exp(-spn)
                spn = work.tile([P, chunk_f], f32, tag="spn")
                nc.scalar.activation(spn[:], x[:], Act.Exp, scale=-1.0)
                nc.scalar.activation(spn[:], spn[:], Act.Ln, bias=1.0)
                p16 = bfp.tile([P, chunk_f], bf16, tag="p16")
                nc.scalar.activation(p16[:], spn[:], Act.Exp, scale=-1.0)

                # ---- d16 = p16 - y16 (bf16) with fused sum(d)
                # (tensor_tensor_reduce is broken on HW; stt works)
                y16 = bfp.tile([P, chunk_f], bf16, tag="y16")
                nc.vector.tensor_scalar(out=y16[:], in0=y[:], scalar1=0.0,
                                        scalar2=None, op0=Alu.add)
                d16 = bfp.tile([P, chunk_f], bf16, tag="d16")
                nc.vector.scalar_tensor_tensor(
                    out=d16[:], in0=p16[:], scalar=0.0, in1=y16[:],
                    op0=Alu.add, op1=Alu.subtract,
                    accum_out=acc[:, col : col + 1])

                # ---- threshold masks (+ fused count/sign sums), written
                # block-interleaved per PSUM bank group so one matmul can
                # consume 4 thresholds: mg[k][p, blk, slot, col]
                mg = [bfp.tile([P, nblk, min(4, NT - 4 * k), 128], bf16,
                               tag=f"mg{k}", name=f"mg{k}_{c}")
                      for k in range(n_banks)]
                for i in range(NT):
                    g = G_OF[i]
                    out_ap = mg[g // 4][:, :, g % 4, :]
                    if i in ACT_SIGN:
                        nc.scalar.activation(
                            out_ap, spn[:], Act.Sign, scale=-1.0,
                            bias=bias_t[:, i : i + 1],
                            accum_out=acc[:, col + 5 + i : col + 6 + i])
                    else:
                        # exact mask: p >= b_i <=> spn < L_i (count via PE)
                        nc.vector.tensor_scalar(
                            out=out_ap, in0=spn[:], scalar1=LTHRESH[i],
                            scalar2=None, op0=Alu.is_lt)

                # ---- PE: ps_bank[k] += d_blk^T @ [4 masks]  (diag = sums);
                # plus ones^T @ bank0-masks for the DVE-group counts
                for b in range(nblk):
                    bs = slice(b * 128, (b + 1) * 128)
                    last = c == n_chunk - 1 and b == nblk - 1
                    for k in range(n_banks):
                        nc.tensor.matmul(ps_banks[k][:],
                                         d16[:, bs],
                                         mg[k][:, b, :, :],
                                         start=False, stop=last,
                                         skip_group_check=True)
                    nc.tensor.matmul(ps_cnt[:], ones16[:],
                                     mg[0][:, b, :n_cnt, :],
                                     start=False, stop=last,
                                     skip_group_check=True)

                # ---- seg loss partials
                ueps = work.tile([P, chunk_f], f32, tag="ueps")
                nc.vector.tensor_scalar(out=ueps[:], in0=u[:], scalar1=1e-6,
                                        scalar2=None, op0=Alu.add)
                w = work.tile([P, chunk_f], f32, tag="w")
                nc.vector.reciprocal_approx_fast(out=w[:], in_=ueps[:])
                scrA = work.tile([P, chunk_f], f32, tag="scrA")
                nc.vector.affine_mul_reduce(
                    out=scrA[:], accum_out=acc[:, col + 1 : col + 2],
                    in0=spn[:], in1=w[:], scale=1.0, bias=0.0)
                # t1 = (y-1)*x in one fused DVE op (gpsimd ts is ~12x slower)
                y1 = work.tile([P, chunk_f], f32, tag="y1")
                nc.vector.scalar_tensor_tensor(
                    out=y1[:], in0=y[:], scalar=1.0, in1=x[:],
                    op0=Alu.subtract, op1=Alu.mult)
                scrB = work.tile([P, chunk_f], f32, tag="scrB")
                nc.vector.affine_mul_reduce(
                    out=scrB[:], accum_out=acc[:, col + 2 : col + 3],
                    in0=y1[:], in1=w[:], scale=1.0, bias=0.0)

                # ---- evidential: sum(a) and sum(e)
                ae = work.tile([P, chunk_f], f32, tag="ae")
                nc.gpsimd.tensor_tensor(out=ae[:], in0=a[:], in1=e[:],
                                        op=Alu.add)
                nc.vector.tensor_scalar(out=ae[:], in0=ae[:], scalar1=0.0,
                                        scalar2=None, op0=Alu.add, op1=Alu.add,
                                        accum_out=acc[:, col + 3 : col + 4])

            # ---- diag extraction: PSUM -> SBUF via ACT copy (DVE PSUM reads
            # are slow), then acc <- sum_n sb[m,n]*I[m,n]
            for k in range(n_banks):
                w_bank = min(512, (NT - 4 * k) * 128)
                sb = stat.tile([P, w_bank], f32, tag=f"sbps{k}",
                               name=f"sbps{k}")
                nc.scalar.copy(sb[:], ps_banks[k][:])
                for j in range(w_bank // 128):
                    i = GROUP_ORDER[4 * k + j]
                    scr_d = stat.tile([P, 128], f32, tag=f"scrd{i}",
                                      name=f"scrd{i}")
                    nc.vector.scalar_tensor_tensor(
                        out=scr_d[:], in0=sb[:, j * 128 : (j + 1) * 128],
                        scalar=1.0, in1=id_t[:], op0=Alu.mult, op1=Alu.mult,
                        accum_out=acc[:, CPC * n_chunk + i :
                                      CPC * n_chunk + i + 1])
            # ---- DVE-group counts: every row of ps_cnt slot s equals the
            # per-column count sums; row-sum = c_i replicated per partition
            sbc = stat.tile([P, n_cnt * 128], f32, tag="sbc")
            nc.scalar.copy(sbc[:], ps_cnt[:])
            for s in range(n_cnt):
                scr_c = stat.tile([P, 128], f32, tag=f"scrc{s}",
                                  name=f"scrc{s}")
                nc.vector.scalar_tensor_tensor(
                    out=scr_c[:], in0=sbc[:, s * 128 : (s + 1) * 128],
                    scalar=1.0, in1=ones16[:], op0=Alu.mult, op1=Alu.mult,
                    accum_out=acc[:, CNT_COL + s : CNT_COL + s + 1])

            nc.sync.dma_start(acc_out[:], acc[:])

    nc.compile()
    return nc


# ---------------- host-side combine ----------------

def combine_host(acc_list, img_f=IMG_F, chunk_f=CHUNK_F, mahal_n=MAHAL_N):
    """acc_list: per-core [128, ACC_COLS] f32 arrays -> 4 f32 scalars."""
    n_chunk = img_f // chunk_f
    n_img = float(P * img_f)
    seg_num = 0.0
    evid_num = 0.0
    cal_imgs = []
    ood_imgs = []
    for arr in acc_list:
        cs = arr.astype(np.float64).sum(axis=0)   # [ACC_COLS]
        sum_d = 0.0
        A = 0.0
        Bv = 0.0
        sum_ae = 0.0
        tsum = np.zeros(NT)
        for c in range(n_chunk):
            col = c * CPC
            sum_d += cs[col]
            A += cs[col + 1]
            Bv += cs[col + 2]
            sum_ae += cs[col + 3]
            tsum += cs[col + 5 : col + 5 + NT]
        diag = cs[CPC * n_chunk : CPC * n_chunk + NT]
        ood = cs[CPC * n_chunk + NT]
        cnt_col = CPC * n_chunk + NT + 1
        # ones-matmul counts are replicated across the 128 partitions
        pe_cnt = cs[cnt_col : cnt_col + len(DVE_MASK)] / float(P)

        # cumulative counts / d-sums per threshold
        ccount = np.empty(NT + 2)
        dcum = np.empty(NT + 2)
        ccount[0] = n_img
        dcum[0] = sum_d
        for i in range(NT):
            if i in ACT_SIGN:
                ccount[i + 1] = (tsum[i] + n_img) / 2.0
                dcum[i + 1] = (diag[i] + sum_d) / 2.0
            else:
                ccount[i + 1] = pe_cnt[DVE_MASK.index(i)]
                dcum[i + 1] = diag[i]
        ccount[NT + 1] = 0.0
        dcum[NT + 1] = 0.0

        n_bin = ccount[:-1] - ccount[1:]          # [10]
        d_bin = dcum[:-1] - dcum[1:]
        valid = n_bin > 0
        safe = np.where(valid, n_bin, 1.0)
        err = np.abs(d_bin / safe)
        n_valid = valid.sum()
        cal_imgs.append((err * valid).sum() / max(n_valid, 1.0)
                        if n_valid > 0 else 0.0)

        seg_num += A - Bv
        evid_num += sum_ae
        ood_imgs.append(ood / float(mahal_n))

    n_cores = len(acc_list)
    seg = seg_num / (n_cores * n_img)
    cal = float(np.mean(cal_imgs))
    uncert = float(np.mean(ood_imgs)) + evid_num / (n_cores * n_img)
    total = seg + LAMBDA_CAL * cal + LAMBDA_UNCERT * uncert
    return np.array([total, seg, cal, uncert], dtype=np.float32)


# ---------------- public entry point ----------------

_NC_CACHE = {}


def _get_nc():
    if "nc" not in _NC_CACHE:
        _NC_CACHE["nc"] = build_nc()
    return _NC_CACHE["nc"]


def kernel(pred_masks, pseudo_labels, total_uncertainty,
           aleatoric_uncertainty, epistemic_uncertainty, mahal_distances):
    pm = np.ascontiguousarray(np.asarray(pred_masks, dtype=np.float32))
    pl = np.ascontiguousarray(np.asarray(pseudo_labels, dtype=np.float32))
    tu = np.ascontiguousarray(np.asarray(total_uncertainty, dtype=np.float32))
    au = np.ascontiguousarray(np.asarray(aleatoric_uncertainty, dtype=np.float32))
    eu = np.ascontiguousarray(np.asarray(epistemic_uncertainty, dtype=np.float32))
    mh = np.ascontiguousarray(np.asarray(mahal_distances, dtype=np.float32))

    nc = _get_nc()
    eye = np.eye(P, dtype=np.float32)
    in_maps = []
    for b in range(N_CORES):
        in_maps.append({
            "pm": pm[b].reshape(P, IMG_F),
            "pl": pl[b].reshape(P, IMG_F),
            "tu": tu[b].reshape(P, IMG_F),
            "au": au[b].reshape(P, IMG_F),
            "eu": eu[b].reshape(P, IMG_F),
            "mh": mh[b].reshape(P, MAHAL_F),
            "ident": eye,
        })
    res = run_bass_kernel_spmd(nc, in_maps, core_ids=list(range(N_CORES)))
    return combine_host([res.results[b]["acc"] for b in range(N_CORES)])


# revision 31
# speedup vs baseline: 2.0009x; 1.0775x over previous
"""Trainium2 Bass kernel for nn_CalibrationAwareLoss.

Strategy (pure data parallel, 1 image per NeuronCore):
  - ACT (ScalarE, natural_log_exp table): sp(-x) = Ln(1+Exp(-x)),
    p16 = Exp(-sp(-x)) as bf16, and K_ACT threshold Sign passes that
    produce +-1 masks (bf16) with fused count accumulation.
  - DVE: bf16 d16 = p16-y16 with fused sum, remaining threshold masks via
    fused is_ge*ones stt with count accumulation, w = 1/(u+1e-6) via
    reciprocal_approx_fast, seg-loss partial sums via affine_mul_reduce.
  - PE (TensorE): per-threshold masked sums  sum(d * mask)  computed as
    diag(d_blk^T @ mask_blk) accumulated in PSUM over all 128-col blocks;
    diagonals extracted with one fused stt (mult identity + accum) each.
  - GPSIMD: t1 = (y-1)*x and (a+e) elementwise products for the seg /
    evidential terms.
Each core DMAs a [128, 128] f32 tile of raw partial sums; the host
combines them in float64 (cheap O(100) work) into the 4 scalar outputs.
"""
import sys

if "/opt/trn_rl_repo" not in sys.path:
    sys.path.insert(0, "/opt/trn_rl_repo")

import numpy as np

import concourse.bacc as bacc
import concourse.mybir as mybir
import concourse.tile as tile
from concourse.bass_utils import run_bass_kernel_spmd

# Pin every activation to the natural_log_exp_and_others table (has Exp, Ln,
# Sign, Copy, Identity, MemsetZero) so the table-load pass emits one load
# instead of thrashing between tables (~1.3us per reload on HW).
_orig_get_tables = bacc.get_activation_tables


def _pinned_tables(arch):
    t = _orig_get_tables(arch)
    return {name: (funcs if name == "natural_log_exp_and_others" else set())
            for name, funcs in t.items()}


bacc.get_activation_tables = _pinned_tables

f32 = mybir.dt.float32
bf16 = mybir.dt.bfloat16
Alu = mybir.AluOpType
Act = mybir.ActivationFunctionType

# ---------------- problem constants (hardcoded per spec) ----------------
B, H, W = 8, 1024, 1024
P = 128
IMG_F = H * W // P          # 8192
MAHAL_N = 4096
MAHAL_F = MAHAL_N // P      # 32
N_CORES = 8
LAMBDA_CAL = 1.0
LAMBDA_UNCERT = 0.1

# f32 bit patterns of jnp.linspace(0,1,11)[1:10]
_BOUND_BITS = [0x3DCCCCCD, 0x3E4CCCCD, 0x3E99999A, 0x3ECCCCCD, 0x3F000000,
               0x3F19999A, 0x3F333333, 0x3F4CCCCD, 0x3F666667]
THRESH = [float(np.uint32(b).view(np.float32)) for b in _BOUND_BITS]
NT = len(THRESH)            # 9
# log-space thresholds: p >= b  <=>  spn = softplus(-x) <= -ln(b)
LTHRESH = [float(np.float32(-np.log(np.float64(t)))) for t in THRESH]

# Thresholds handled by ACT Sign passes compare spn (full f32 precision,
# mask = sign(L_i - spn) in {-1,+1}); the rest compare spn on DVE
# (mask = (spn < L_i) in {0,1}).  ACT gets the middle (densest) bins.
ACT_SIGN = [2, 3, 4, 5, 6]
DVE_MASK = [i for i in range(NT) if i not in ACT_SIGN]
# PSUM bank grouping: DVE-mask thresholds share bank 0 so ONE extra
# ones-matmul stream per block yields all their counts (the DVE mask ops
# then skip the slow fused-reduce path); ACT thresholds' counts ride free
# on their activation accum_out.
GROUP_ORDER = DVE_MASK + ACT_SIGN           # threshold index by (bank,slot)
G_OF = {i: g for g, i in enumerate(GROUP_ORDER)}

CHUNK_F = 1024
# accumulator column layout, per chunk: [sum_d, A, B, sum_a, sum_e, t0..t8];
# image-level tail: diag[NT], ood, dve-group counts[len(DVE_MASK)]
CPC = 5 + NT                # 14 columns per chunk
ACC_COLS = 128


def build_nc(img_f=IMG_F, chunk_f=CHUNK_F, mahal_f=MAHAL_F):
    n_chunk = img_f // chunk_f
    nblk = chunk_f // 128
    CNT_COL = CPC * n_chunk + NT + 1
    assert CNT_COL + len(DVE_MASK) <= ACC_COLS

    nc = bacc.Bacc("TRN2", target_bir_lowering=False, debug=False,
                   num_devices=N_CORES)
    pm = nc.dram_tensor("pm", [P, img_f], f32, kind="ExternalInput")
    pl = nc.dram_tensor("pl", [P, img_f], f32, kind="ExternalInput")
    tu = nc.dram_tensor("tu", [P, img_f], f32, kind="ExternalInput")
    au = nc.dram_tensor("au", [P, img_f], f32, kind="ExternalInput")
    eu = nc.dram_tensor("eu", [P, img_f], f32, kind="ExternalInput")
    mh = nc.dram_tensor("mh", [P, mahal_f], f32, kind="ExternalInput")
    ident = nc.dram_tensor("ident", [P, 128], f32, kind="ExternalInput")
    acc_out = nc.dram_tensor("acc", [P, ACC_COLS], f32, kind="ExternalOutput")

    with tile.TileContext(nc) as tc:
        with (
            tc.tile_pool(name="io", bufs=2) as io,
            tc.tile_pool(name="work", bufs=2) as work,
            tc.tile_pool(name="bfp", bufs=2) as bfp,
            tc.tile_pool(name="stat", bufs=1) as stat,
            tc.tile_pool(name="psum", bufs=1, space="PSUM") as psp,
        ):
            # persistent tiles
            acc = stat.tile([P, ACC_COLS], f32, tag="acc")
            nc.vector.memset(acc[:], 0.0)
            id_t = stat.tile([P, 128], f32, tag="ident")
            nc.sync.dma_start(id_t[:], ident[:])
            zeros_m = stat.tile([P, mahal_f], f32, tag="zeros_m")
            nc.gpsimd.memset(zeros_m[:], 0.0)
            bias_t = stat.tile([P, NT], f32, tag="bias")
            for i in range(NT):
                nc.gpsimd.memset(bias_t[:, i : i + 1], LTHRESH[i])
            eps_t = stat.tile([P, 1], f32, tag="eps")
            nc.gpsimd.memset(eps_t[:], 1e-6)

            # PSUM tiles are padded to a full bank (2KB/partition); pack 4
            # threshold accumulators per bank as 128-col slices.
            n_banks = (NT + 3) // 4
            ps_banks = [psp.tile([128, min(512, (NT - 4 * k) * 128)], f32,
                                 tag=f"psb{k}", name=f"psb{k}")
                        for k in range(n_banks)]
            for k in range(n_banks):
                nc.vector.memset(ps_banks[k][:], 0.0)
            # count accumulator bank for the DVE-mask group (bank 0)
            n_cnt = len(DVE_MASK)
            ps_cnt = psp.tile([128, n_cnt * 128], f32, tag="pscnt",
                              name="pscnt")
            nc.vector.memset(ps_cnt[:], 0.0)
            ones16 = stat.tile([P, 128], bf16, tag="ones16")
            nc.gpsimd.memset(ones16[:], 1.0)

            # mahal: ood partial = sum(relu(mh - 2))
            mh_t = stat.tile([P, mahal_f], f32, tag="mh")
            nc.sync.dma_start(mh_t[:], mh[:])
            scr_m = stat.tile([P, mahal_f], f32, tag="scr_m")
            nc.vector.scalar_tensor_tensor(
                out=scr_m[:], in0=mh_t[:], scalar=2.0, in1=zeros_m[:],
                op0=Alu.subtract, op1=Alu.max,
                accum_out=acc[:, CPC * n_chunk + NT : CPC * n_chunk + NT + 1])

            for c in range(n_chunk):
                sl = slice(c * chunk_f, (c + 1) * chunk_f)
                col = c * CPC

                x = io.tile([P, chunk_f], f32, tag="x")
                y = io.tile([P, chunk_f], f32, tag="y")
                u = io.tile([P, chunk_f], f32, tag="u")
                a = io.tile([P, chunk_f], f32, tag="a")
                e = io.tile([P, chunk_f], f32, tag="e")
                nc.sync.dma_start(x[:], pm[:, sl])
                nc.sync.dma_start(y[:], pl[:, sl])
                nc.sync.dma_start(u[:], tu[:, sl])
                nc.sync.dma_start(a[:], au[:, sl])
                nc.sync.dma_start(e[:], eu[:, sl])

                # ---- ACT chain: e1=Exp(-x); spn=Ln(e1+1); p16=Exp(-spn)
                spn = work.tile([P, chunk_f], f32, tag="spn")
                nc.scalar.activation(spn[:], x[:], Act.Exp, scale=-1.0)
                nc.scalar.activation(spn[:], spn[:], Act.Ln, bias=1.0)
                p16 = bfp.tile([P, chunk_f], bf16, tag="p16")
                nc.scalar.activation(p16[:], spn[:], Act.Exp, scale=-1.0)

                # ---- d16 = p16 - y (bf16 out) with fused sum(d)
                # (tensor_tensor_reduce is broken on HW; stt works)
                d16 = bfp.tile([P, chunk_f], bf16, tag="d16")
                nc.vector.scalar_tensor_tensor(
                    out=d16[:], in0=p16[:], scalar=0.0, in1=y[:],
                    op0=Alu.add, op1=Alu.subtract,
                    accum_out=acc[:, col : col + 1])

                # ---- threshold masks (+ fused count/sign sums), written
                # block-interleaved per PSUM bank group so one matmul can
                # consume 4 thresholds: mg[k][p, blk, slot, col]
                mg = [bfp.tile([P, nblk, min(4, NT - 4 * k), 128], bf16,
                               tag=f"mg{k}", name=f"mg{k}_{c}")
                      for k in range(n_banks)]
                for i in range(NT):
                    g = G_OF[i]
                    out_ap = mg[g // 4][:, :, g % 4, :]
                    if i in ACT_SIGN:
                        nc.scalar.activation(
                            out_ap, spn[:], Act.Sign, scale=-1.0,
                            bias=bias_t[:, i : i + 1],
                            accum_out=acc[:, col + 5 + i : col + 6 + i])
                    else:
                        # exact mask: p >= b_i <=> spn < L_i (count via PE)
                        nc.vector.tensor_scalar(
                            out=out_ap, in0=spn[:], scalar1=LTHRESH[i],
                            scalar2=None, op0=Alu.is_lt)

                # ---- PE: ps_bank[k] += d_blk^T @ [4 masks]  (diag = sums);
                # plus ones^T @ bank0-masks for the DVE-group counts
                for b in range(nblk):
                    bs = slice(b * 128, (b + 1) * 128)
                    last = c == n_chunk - 1 and b == nblk - 1
                    for k in range(n_banks):
                        nc.tensor.matmul(ps_banks[k][:],
                                         d16[:, bs],
                                         mg[k][:, b, :, :],
                                         start=False, stop=last,
                                         skip_group_check=True)
                    nc.tensor.matmul(ps_cnt[:], ones16[:],
                                     mg[0][:, b, :n_cnt, :],
                                     start=False, stop=last,
                                     skip_group_check=True)

                # ---- seg loss partials (u+1e-6 on ACT to offload DVE)
                ueps = work.tile([P, chunk_f], f32, tag="ueps")
                nc.scalar.activation(ueps[:], u[:], Act.Identity,
                                     bias=eps_t[:])
                w = work.tile([P, chunk_f], f32, tag="w")
                nc.vector.reciprocal_approx_fast(out=w[:], in_=ueps[:])
                scrA = work.tile([P, chunk_f], f32, tag="scrA")
                nc.vector.affine_mul_reduce(
                    out=scrA[:], accum_out=acc[:, col + 1 : col + 2],
                    in0=spn[:], in1=w[:], scale=1.0, bias=0.0)
                # t1 = (y-1)*x in one fused DVE op (gpsimd ts is ~12x slower)
                y1 = work.tile([P, chunk_f], f32, tag="y1")
                nc.vector.scalar_tensor_tensor(
                    out=y1[:], in0=y[:], scalar=1.0, in1=x[:],
                    op0=Alu.subtract, op1=Alu.mult)
                scrB = work.tile([P, chunk_f], f32, tag="scrB")
                nc.vector.affine_mul_reduce(
                    out=scrB[:], accum_out=acc[:, col + 2 : col + 3],
                    in0=y1[:], in1=w[:], scale=1.0, bias=0.0)

                # ---- evidential: sum(a) and sum(e)
                ae = work.tile([P, chunk_f], f32, tag="ae")
                nc.gpsimd.tensor_tensor(out=ae[:], in0=a[:], in1=e[:],
                                        op=Alu.add)
                nc.vector.tensor_scalar(out=ae[:], in0=ae[:], scalar1=0.0,
                                        scalar2=None, op0=Alu.add, op1=Alu.add,
                                        accum_out=acc[:, col + 3 : col + 4])

            # ---- diag extraction: PSUM -> SBUF via ACT copy (DVE PSUM reads
            # are slow), then acc <- sum_n sb[m,n]*I[m,n]
            for k in range(n_banks):
                w_bank = min(512, (NT - 4 * k) * 128)
                sb = stat.tile([P, w_bank], f32, tag=f"sbps{k}",
                               name=f"sbps{k}")
                nc.scalar.copy(sb[:], ps_banks[k][:])
                for j in range(w_bank // 128):
                    i = GROUP_ORDER[4 * k + j]
                    scr_d = stat.tile([P, 128], f32, tag=f"scrd{i}",
                                      name=f"scrd{i}")
                    nc.vector.scalar_tensor_tensor(
                        out=scr_d[:], in0=sb[:, j * 128 : (j + 1) * 128],
                        scalar=1.0, in1=id_t[:], op0=Alu.mult, op1=Alu.mult,
                        accum_out=acc[:, CPC * n_chunk + i :
                                      CPC * n_chunk + i + 1])
            # ---- DVE-group counts: every row of ps_cnt slot s equals the
            # per-column count sums; row-sum = c_i replicated per partition
            sbc = stat.tile([P, n_cnt * 128], f32, tag="sbc")
            nc.scalar.copy(sbc[:], ps_cnt[:])
            for s in range(n_cnt):
                scr_c = stat.tile([P, 128], f32, tag=f"scrc{s}",
                                  name=f"scrc{s}")
                nc.vector.scalar_tensor_tensor(
                    out=scr_c[:], in0=sbc[:, s * 128 : (s + 1) * 128],
                    scalar=1.0, in1=ones16[:], op0=Alu.mult, op1=Alu.mult,
                    accum_out=acc[:, CNT_COL + s : CNT_COL + s + 1])

            nc.sync.dma_start(acc_out[:], acc[:])

    nc.compile()
    return nc


# ---------------- host-side combine ----------------

def combine_host(acc_list, img_f=IMG_F, chunk_f=CHUNK_F, mahal_n=MAHAL_N):
    """acc_list: per-core [128, ACC_COLS] f32 arrays -> 4 f32 scalars."""
    n_chunk = img_f // chunk_f
    n_img = float(P * img_f)
    seg_num = 0.0
    evid_num = 0.0
    cal_imgs = []
    ood_imgs = []
    for arr in acc_list:
        cs = arr.astype(np.float64).sum(axis=0)   # [ACC_COLS]
        sum_d = 0.0
        A = 0.0
        Bv = 0.0
        sum_ae = 0.0
        tsum = np.zeros(NT)
        for c in range(n_chunk):
            col = c * CPC
            sum_d += cs[col]
            A += cs[col + 1]
            Bv += cs[col + 2]
            sum_ae += cs[col + 3]
            tsum += cs[col + 5 : col + 5 + NT]
        diag = cs[CPC * n_chunk : CPC * n_chunk + NT]
        ood = cs[CPC * n_chunk + NT]
        cnt_col = CPC * n_chunk + NT + 1
        # ones-matmul counts are replicated across the 128 partitions
        pe_cnt = cs[cnt_col : cnt_col + len(DVE_MASK)] / float(P)

        # cumulative counts / d-sums per threshold
        ccount = np.empty(NT + 2)
        dcum = np.empty(NT + 2)
        ccount[0] = n_img
        dcum[0] = sum_d
        for i in range(NT):
            if i in ACT_SIGN:
                ccount[i + 1] = (tsum[i] + n_img) / 2.0
                dcum[i + 1] = (diag[i] + sum_d) / 2.0
            else:
                ccount[i + 1] = pe_cnt[DVE_MASK.index(i)]
                dcum[i + 1] = diag[i]
        ccount[NT + 1] = 0.0
        dcum[NT + 1] = 0.0

        n_bin = ccount[:-1] - ccount[1:]          # [10]
        d_bin = dcum[:-1] - dcum[1:]
        valid = n_bin > 0
        safe = np.where(valid, n_bin, 1.0)
        err = np.abs(d_bin / safe)
        n_valid = valid.sum()
        cal_imgs.append((err * valid).sum() / max(n_valid, 1.0)
                        if n_valid > 0 else 0.0)

        seg_num += A - Bv
        evid_num += sum_ae
        ood_imgs.append(ood / float(mahal_n))

    n_cores = len(acc_list)
    seg = seg_num / (n_cores * n_img)
    cal = float(np.mean(cal_imgs))
    uncert = float(np.mean(ood_imgs)) + evid_num / (n_cores * n_img)
    total = seg + LAMBDA_CAL * cal + LAMBDA_UNCERT * uncert
    return np.array([total, seg, cal, uncert], dtype=np.float32)


# ---------------- public entry point ----------------

_NC_CACHE = {}


def _get_nc():
    if "nc" not in _NC_CACHE:
        _NC_CACHE["nc"] = build_nc()
    return _NC_CACHE["nc"]


def kernel(pred_masks, pseudo_labels, total_uncertainty,
           aleatoric_uncertainty, epistemic_uncertainty, mahal_distances):
    pm = np.ascontiguousarray(np.asarray(pred_masks, dtype=np.float32))
    pl = np.ascontiguousarray(np.asarray(pseudo_labels, dtype=np.float32))
    tu = np.ascontiguousarray(np.asarray(total_uncertainty, dtype=np.float32))
    au = np.ascontiguousarray(np.asarray(aleatoric_uncertainty, dtype=np.float32))
    eu = np.ascontiguousarray(np.asarray(epistemic_uncertainty, dtype=np.float32))
    mh = np.ascontiguousarray(np.asarray(mahal_distances, dtype=np.float32))

    nc = _get_nc()
    eye = np.eye(P, dtype=np.float32)
    in_maps = []
    for b in range(N_CORES):
        in_maps.append({
            "pm": pm[b].reshape(P, IMG_F),
            "pl": pl[b].reshape(P, IMG_F),
            "tu": tu[b].reshape(P, IMG_F),
            "au": au[b].reshape(P, IMG_F),
            "eu": eu[b].reshape(P, IMG_F),
            "mh": mh[b].reshape(P, MAHAL_F),
            "ident": eye,
        })
    res = run_bass_kernel_spmd(nc, in_maps, core_ids=list(range(N_CORES)))
    return combine_host([res.results[b]["acc"] for b in range(N_CORES)])
